# revision 23
# baseline (speedup 1.0000x reference)
"""Trainium2 Bass kernel for the PhysicsInformedLoss problem.

Data-parallel across 8 NeuronCores: each core processes a 512-row shard of the
[4096, 8192] inputs and produces small partial sums; the host combines them
with O(B) numpy work (final divisions, per-row physics, last-index gathers).

Device-side strategy per core (4 row-tiles x 4 column-chunks of 2048):
  ACT    : mask cast u8->fp16, log(1-Q), square(P-T), |S2|, boundary relu
           (+accum)
  GPSIMD : t1 = P-T, S2 = D[s+1]-D[s]   (fp32 tensor_tensor, runs parallel)
  DVE    : D = P[s+1]-P[s] (fp32), per-row sum of D*mask (physics accum),
           vp/vt mask products (fp16 2x), min(D,0) (fp32 tensor_scalar 2x),
           per-row 32-col mask group counts (drives last-valid + row counts)
  PE     : all global masked sums as fp16 Gram matmuls (block-diagonal traces)
           plus ones-colsums for vp/vt totals, accumulated in PSUM
"""

import sys

_REPO = "/opt/trn_rl_repo"
if _REPO not in sys.path:
    sys.path.insert(0, _REPO)

from contextlib import ExitStack

import numpy as np

import concourse.bacc as bacc
import concourse.mybir as mybir
import concourse.tile as tile
from concourse.bass_utils import run_bass_kernel_spmd

F32 = mybir.dt.float32
FP16 = mybir.dt.float16
U8 = mybir.dt.uint8

B, S = 4096, 8192
NCORES = 8
BSH = B // NCORES          # rows per core
P = 128                    # partitions per row-tile
THR = 200.0
EPS = 1e-7
G = 32                     # mask group size for last-valid search

QI_VD, QI_BND = 0, 1       # rowacc quantity indices


def build_program(bsh=BSH, s=S, f=2048):
    """Build the per-core Bass program. All shapes compile-time constant."""
    nrt = bsh // P
    nch = s // f
    nblk = f // 128
    ng = f // G            # groups per chunk

    nc = bacc.Bacc("TRN2", target_bir_lowering=False, num_devices=NCORES)
    p_in = nc.dram_tensor("p", [bsh, s], F32, kind="ExternalInput").ap()
    t_in = nc.dram_tensor("t", [bsh, s], F32, kind="ExternalInput").ap()
    q_in = nc.dram_tensor("q", [bsh, s], F32, kind="ExternalInput").ap()
    m_in = nc.dram_tensor("m", [bsh, s], U8, kind="ExternalInput").ap()
    # grams rows 0..127: [:,0:128] e2-gram, [:,128:256] bce-gram,
    # [:,256:384] mono-gram, [:,384:512] smooth-gram;
    # row 0 cols 512:1024 = vps colsums, row 1 cols 512:1024 = vts colsums
    grams = nc.dram_tensor("grams", [P, 1536], F32, kind="ExternalOutput").ap()
    rowacc = nc.dram_tensor("rowacc", [nrt, P, 2 * nch], F32,
                            kind="ExternalOutput").ap()
    grpcnt = nc.dram_tensor("grpcnt", [nrt, P, nch * ng], F32,
                            kind="ExternalOutput").ap()

    n_eb = nrt * nch * nblk
    n_mono = nrt * sum((min(f, s - 1 - c * f) + 127) // 128 for c in range(nch))
    n_sm = nrt * sum((min(f, s - 2 - c * f) + 127) // 128 for c in range(nch))
    cs = min(512, f)       # colsum block width
    n_vps = nrt * sum((min(f, s - 1 - c * f) + cs - 1) // cs for c in range(nch))
    n_vts = nrt * sum((min(f, s - 2 - c * f) + cs - 1) // cs for c in range(nch))

    with tile.TileContext(nc) as tc, ExitStack() as ctx:
        singles = ctx.enter_context(tc.tile_pool(name="singles", bufs=1))
        inp = ctx.enter_context(tc.tile_pool(name="inp", bufs=3))
        work = ctx.enter_context(tc.tile_pool(name="work", bufs=2))
        accp = ctx.enter_context(tc.tile_pool(name="accp", bufs=2))
        psum = ctx.enter_context(tc.tile_pool(name="psum", bufs=1, space="PSUM"))

        cneg_thr = singles.tile([P, 1], F32)
        nc.vector.memset(cneg_thr, -THR)
        ones_h = singles.tile([P, 1], FP16)
        nc.vector.memset(ones_h, 1.0)

        ps_eb = psum.tile([P, 256], F32)
        ps_mono = psum.tile([P, 128], F32)
        ps_sm = psum.tile([P, 128], F32)
        ps_vps = psum.tile([1, cs], F32)
        ps_vts = psum.tile([1, cs], F32)
        mm_cnt = {"eb": 0, "mono": 0, "sm": 0, "vps": 0, "vts": 0}
        mm_tot = {"eb": n_eb, "mono": n_mono, "sm": n_sm, "vps": n_vps,
                  "vts": n_vts}

        def mm(tag, out_ap, lhsT, rhs):
            i = mm_cnt[tag]
            nc.tensor.matmul(out_ap, lhsT, rhs,
                             start=(i == 0), stop=(i == mm_tot[tag] - 1))
            mm_cnt[tag] = i + 1

        for rt in range(nrt):
            acc = accp.tile([P, 2 * nch], F32)
            grp = accp.tile([P, nch, ng], F32)

            for c in range(nch):
                c0 = c * f
                last_chunk = c == nch - 1
                w = f if last_chunk else f + 2    # loaded columns
                pw = min(f, s - 1 - c0)           # pair outputs this chunk
                tw = min(f, s - 2 - c0)           # triple outputs this chunk
                dw = pw if last_chunk else pw + 1  # D columns computed

                rows = slice(rt * P, (rt + 1) * P)
                pt = inp.tile([P, f + 2], F32, tag="pt")
                tt = inp.tile([P, f], F32, tag="tt")
                qt = inp.tile([P, f], F32, tag="qt")
                mt = inp.tile([P, f + 2], U8, tag="mt")
                nc.sync.dma_start(out=pt[:, :w], in_=p_in[rows, c0:c0 + w])
                nc.sync.dma_start(out=tt[:, :f], in_=t_in[rows, c0:c0 + f])
                nc.sync.dma_start(out=qt[:, :f], in_=q_in[rows, c0:c0 + f])
                nc.sync.dma_start(out=mt[:, :w], in_=m_in[rows, c0:c0 + w])

                mf = work.tile([P, f + 2], FP16, tag="mf")
                t1 = work.tile([P, f], FP16, tag="t1")
                sqlg = work.tile([P, 2, f], FP16, tag="sqlg")
                d = work.tile([P, f + 1], F32, tag="d")
                xr = work.tile([P, f], FP16, tag="xr")
                s2 = work.tile([P, f], FP16, tag="s2")
                abs2 = work.tile([P, f], FP16, tag="abs2")
                vp = work.tile([P, f + 1], FP16, tag="vp")
                vt = work.tile([P, f], FP16, tag="vt")
                vdo = work.tile([P, f], FP16, tag="vdo", bufs=1)
                bout = work.tile([P, f], FP16, tag="bout", bufs=1)

                # --- ACT: mask cast u8 -> fp16 ---
                nc.scalar.activation(mf[:, :w], mt[:, :w],
                                     mybir.ActivationFunctionType.Copy)
                # log(1 - Q)
                nc.scalar.activation(sqlg[:, 1, :f], qt[:, :f],
                                     mybir.ActivationFunctionType.Ln,
                                     scale=-1.0, bias=1.0)

                # --- GPSIMD: t1 = P - T ---
                nc.gpsimd.tensor_tensor(out=t1[:, :f], in0=pt[:, :f],
                                        in1=tt[:, :f],
                                        op=mybir.AluOpType.subtract)
                # (P - T)^2 on ACT
                nc.scalar.activation(sqlg[:, 0, :f], t1[:, :f],
                                     mybir.ActivationFunctionType.Square)

                # --- DVE: D[s] = P[s+1] - P[s] (fp32, exact for physics) ---
                nc.vector.tensor_tensor(out=d[:, :dw], in0=pt[:, 1:dw + 1],
                                        in1=pt[:, :dw],
                                        op=mybir.AluOpType.subtract)
                # min(D, 0) = -relu(-D)  (host negates the mono gram)
                nc.vector.tensor_scalar(out=xr[:, :pw], in0=d[:, :pw],
                                        scalar1=0.0, scalar2=None,
                                        op0=mybir.AluOpType.min)
                # vp[s] = m[s+1]*m[s] (fp16 2x)
                nc.vector.tensor_tensor(out=vp[:, :dw], in0=mf[:, 1:dw + 1],
                                        in1=mf[:, :dw],
                                        op=mybir.AluOpType.mult)
                # vt[s] = vp[s+1]*vp[s]
                nc.vector.tensor_tensor(out=vt[:, :tw], in0=vp[:, 1:tw + 1],
                                        in1=vp[:, :tw],
                                        op=mybir.AluOpType.mult)
                # --- GPSIMD: S2[s] = D[s+1] - D[s] ---
                nc.gpsimd.tensor_tensor(out=s2[:, :tw], in0=d[:, 1:tw + 1],
                                        in1=d[:, :tw],
                                        op=mybir.AluOpType.subtract)
                # |S2| on ACT
                nc.scalar.activation(abs2[:, :tw], s2[:, :tw],
                                     mybir.ActivationFunctionType.Abs)

                # --- DVE: per-row sum of D*m0 (physics) ---
                nc.vector.scalar_tensor_tensor(
                    out=vdo[:, :pw], in0=d[:, :pw], scalar=0.0,
                    in1=mf[:, :pw],
                    op0=mybir.AluOpType.bypass, op1=mybir.AluOpType.mult,
                    accum_out=acc[:, QI_VD * nch + c:QI_VD * nch + c + 1])
                # --- DVE: per-row mask counts per 32-col group ---
                nc.vector.tensor_reduce(
                    out=grp[:, c, :], in_=mf[:, :f].rearrange(
                        "p (g e) -> p g e", g=ng),
                    axis=mybir.AxisListType.X, op=mybir.AluOpType.add)
                # --- ACT: boundary sum relu(P - 200) ---
                nc.scalar.activation(
                    bout[:, :f], pt[:, :f],
                    mybir.ActivationFunctionType.Relu, bias=cneg_thr[:, :],
                    accum_out=acc[:, QI_BND * nch + c:QI_BND * nch + c + 1])

                # --- PE Gram accumulations ---
                for b in range(nblk):
                    bs = b * 128
                    mm("eb", ps_eb, mf[:, bs:bs + 128],
                       sqlg[:, :, bs:bs + 128])
                # ragged (narrow) blocks first so a full-width matmul always
                # closes each PSUM accumulation group
                for b in reversed(range((pw + 127) // 128)):
                    bs = b * 128
                    wb = min(128, pw - bs)
                    mm("mono", ps_mono[0:wb, 0:wb],
                       vp[:, bs:bs + wb], xr[:, bs:bs + wb])
                for b in reversed(range((tw + 127) // 128)):
                    bs = b * 128
                    wb = min(128, tw - bs)
                    mm("sm", ps_sm[0:wb, 0:wb],
                       vt[:, bs:bs + wb], abs2[:, bs:bs + wb])
                # vps / vts totals: ones-colsums (exact ranges, no seam fix)
                for b in reversed(range((pw + cs - 1) // cs)):
                    bs = b * cs
                    wb = min(cs, pw - bs)
                    mm("vps", ps_vps[0:1, 0:wb], ones_h[:, :],
                       vp[:, bs:bs + wb])
                for b in reversed(range((tw + cs - 1) // cs)):
                    bs = b * cs
                    wb = min(cs, tw - bs)
                    mm("vts", ps_vts[0:1, 0:wb], ones_h[:, :],
                       vt[:, bs:bs + wb])

            nc.sync.dma_start(out=rowacc[rt], in_=acc[:, :])
            nc.sync.dma_start(out=grpcnt[rt], in_=grp[:, :, :])

        gsb = singles.tile([P, 1536], F32)
        nc.vector.memset(gsb[:, :], 0.0)
        nc.vector.tensor_copy(gsb[:, 0:256], ps_eb[:, :])
        nc.vector.tensor_copy(gsb[:, 256:384], ps_mono[:, :])
        nc.vector.tensor_copy(gsb[:, 384:512], ps_sm[:, :])
        nc.vector.tensor_copy(gsb[0:1, 512:512 + cs], ps_vps[:, :])
        nc.vector.tensor_copy(gsb[0:1, 1024:1024 + cs], ps_vts[:, :])
        nc.sync.dma_start(out=grams[:, :], in_=gsb[:, :])

    for tag in mm_cnt:
        assert mm_cnt[tag] == mm_tot[tag], (tag, mm_cnt[tag], mm_tot[tag])
    nc.compile()
    return nc


_NC_CACHE = {}


def _get_program(key=(BSH, S, 2048)):
    if key not in _NC_CACHE:
        _NC_CACHE[key] = build_program(*key)
    return _NC_CACHE[key]


def host_finalize(predictions, targets, breakdown_prob, valid_mask,
                  voltage, thickness, grams_list, rowacc_list, grpcnt_list,
                  s=S, f=2048):
    """Combine per-core device partials into the 7 reference losses."""
    nch = s // f
    ng = f // G
    b = predictions.shape[0]
    ncores = len(grams_list)
    bsh = b // ncores
    nrt = bsh // P

    e2 = bce_dev = mono_neg = smooth_num = vps = vts = 0.0
    for g in grams_list:
        g = np.asarray(g, np.float64)
        e2 += np.trace(g[:, 0:128])
        bce_dev += np.trace(g[:, 128:256])
        mono_neg += np.trace(g[:, 256:384])
        smooth_num += np.trace(g[:, 384:512])
        vps += g[0, 512:1024].sum()
        vts += g[0, 1024:1536].sum()
    mono_num = -mono_neg  # device summed min(D,0)*vp

    ra = np.concatenate(
        [np.asarray(r, np.float64).reshape(nrt * P, 2, nch)
         for r in rowacc_list], axis=0)          # [B, 2, nch]
    gc = np.concatenate(
        [np.asarray(r, np.float64).reshape(nrt * P, nch * ng)
         for r in grpcnt_list], axis=0)          # [B, nch*ng] group counts

    vm = valid_mask
    rsm = gc.sum(1)                               # mask count per row (exact)
    msum = rsm.sum()
    any_valid = msum > 0
    vd_rows = ra[:, QI_VD, :].sum(1)
    bound_sum = ra[:, QI_BND, :].sum()

    # last valid index per row: last nonzero 32-col group, then scan inside
    ngt = nch * ng
    has_valid = rsm > 0
    rev_nz = gc[:, ::-1] > 0
    last_grp = ngt - 1 - np.argmax(rev_nz, axis=1)        # [B]
    last_grp = np.where(has_valid, last_grp, 0)
    base = last_grp * G
    block = vm[np.arange(b)[:, None], base[:, None] + np.arange(G)[None, :]]
    last_in = G - 1 - np.argmax(block[:, ::-1], axis=1)
    last = np.where(has_valid, base + last_in, -1).astype(np.float64)
    last_idx = np.clip(last, 0, s - 1).astype(np.int64)

    rows = np.arange(b)
    # --- prediction ---
    pred_loss = e2 / max(msum, 1.0) if any_valid else 0.0

    # --- breakdown ---
    t_l = targets[rows, last_idx].astype(np.float64)
    cond = (last < s - 1) | (t_l >= THR * 0.95)
    bt = has_valid & cond
    nbt = float(bt.sum())
    q_l = breakdown_prob[rows, last_idx].astype(np.float64)
    p_c = np.clip(q_l, EPS, 1.0 - EPS)
    corr = np.where(bt, -5.0 * np.log(p_c) + np.log1p(-q_l), 0.0).sum()
    wsum = msum + 4.0 * nbt
    breakdown_loss = (-bce_dev + corr) / max(wsum, 1.0) if any_valid else 0.0

    # --- monotonic / smoothness ---
    mono_loss = mono_num / max(vps, 1.0) if vps > 0 else 0.0
    smooth_loss = smooth_num / max(vts, 1.0) if vts > 0 else 0.0

    # --- physics ---
    counts = rsm - vm[:, s - 1]
    avg_rate = vd_rows / np.maximum(counts, 1.0)
    pos = avg_rate > 0
    npos = float(pos.sum())
    log_rate = np.log(np.maximum(avg_rate, 1e-8))
    efield = np.abs(voltage.astype(np.float64)) / thickness.astype(np.float64) * 1e-7
    log_ff = np.log(np.maximum(np.exp(0.1 * efield), 1e-8))
    phys_mse = (pos * (log_rate - log_ff) ** 2).sum() / max(npos, 1.0)
    m1sum = msum - float(vm[:, 0].sum())
    physics_loss = phys_mse if (m1sum > 0 and npos > 0) else 0.0

    # --- boundary ---
    boundary_loss = bound_sum / (b * s) * 0.1

    total = (1.0 * pred_loss + 0.5 * breakdown_loss + 0.2 * mono_loss
             + 0.1 * smooth_loss + 0.3 * physics_loss + boundary_loss)
    return np.array([pred_loss, breakdown_loss, mono_loss, smooth_loss,
                     physics_loss, boundary_loss, total], dtype=np.float32)


_LAST_EXEC_NS = None


def kernel(predictions, targets, breakdown_prob, valid_mask, voltage,
           thickness, _trace=False):
    global _LAST_EXEC_NS
    nc = _get_program()
    predictions = np.ascontiguousarray(predictions, dtype=np.float32)
    targets = np.ascontiguousarray(targets, dtype=np.float32)
    breakdown_prob = np.ascontiguousarray(breakdown_prob, dtype=np.float32)
    mask_u8 = np.ascontiguousarray(valid_mask, dtype=bool).view(np.uint8)
    voltage = np.asarray(voltage, dtype=np.float32)
    thickness = np.asarray(thickness, dtype=np.float32)

    in_maps = []
    for k in range(NCORES):
        r = slice(k * BSH, (k + 1) * BSH)
        in_maps.append({
            "p": predictions[r], "t": targets[r],
            "q": breakdown_prob[r], "m": mask_u8[r],
        })
    res = run_bass_kernel_spmd(nc, in_maps, core_ids=list(range(NCORES)),
                               trace=_trace)
    _LAST_EXEC_NS = res.exec_time_ns
    grams_list = [res.results[k]["grams"] for k in range(NCORES)]
    rowacc_list = [res.results[k]["rowacc"] for k in range(NCORES)]
    grpcnt_list = [res.results[k]["grpcnt"] for k in range(NCORES)]
    return host_finalize(predictions, targets, breakdown_prob,
                         np.asarray(valid_mask, dtype=bool), voltage,
                         thickness, grams_list, rowacc_list, grpcnt_list)
